# revision 2
# baseline (speedup 1.0000x reference)
"""Trainium2 Bass kernel v2 for nn_MultiHeadBindingAttention.

Math (per (b,h) pair; T=2048, HD=1024):
    z[t,s]   = c_h * raw[t,s],  raw = x @ (x*sgn_qk)^T   (symmetric)
    attn     = causal(sigmoid(z)) = 0.5*tril + dev,  dev = causal(sigmoid(z) - 0.5)
    out[t,:] = attn @ xv,  xv = x * v_bind

Device computes:  raw_out = SNET * ( dev @ xv  +  0.5 * triu_diagblock @ xv )
  - dev tiles in fp8e4 (scaled by SDEV), DoubleRow-packed; all AV matmuls
    (dev and the intra-128-block 0.5*triangle term) run fp8 DoubleRow with
    the same moving operand xv8 = SV*xv, so the PE never switches modes
    inside the AV phase.
  - the inter-block part of the 0.5-term (0.5 * sum_{s < block} xv[s,:]) is
    a per-block bias row the host adds in the epilogue.
  - scores matmuls in fp8 DoubleRowSwInterleave (software-interleaved
    stationary -> faster weight path).

Sharding: 16 (b,h) pairs data-parallel over 8 cores, 2 pairs/core.
"""

import numpy as np

import concourse.bacc as bacc
import concourse.tile as tile
from concourse import mybir
from concourse.bass_utils import run_bass_kernel_spmd

B, T, D = 4, 2048, 4096
H, HD = 4, 1024
N_CORES = 8
PAIRS = 2
P = 128
TB = 512
NTB = T // TB                  # 4 strips
NSC = T // P                   # 16 s-chunks
NPC = NSC // 2                 # 8 s-pair chunks (DoubleRow)
DRCH = HD // (2 * P)           # 4 score contraction chunks of 256

F32 = mybir.dt.float32
F16 = mybir.dt.float16
FP8 = mybir.dt.float8e4

SDEV = 128.0                   # dev tile scale
SV = 256.0                     # xv fp8 scale
SNET = SDEV * SV               # net output scale (= 32768)

SIGMOID = True                 # True: real sigmoid + DVE affine; False: linear Copy
SCORES_SWI = True              # SwInterleave stationary for scores
TRIL8 = True                   # fp8-DR intra-block triangle (else fp16 + xv16)

_program_cache = None


def _build_program(reps=1, no_av=False, no_scores=False,
                   sigmoid=SIGMOID, swi=SCORES_SWI, tril8=TRIL8,
                   load_once=False, no_out=False):
    nc = bacc.Bacc(trn_type="TRN2", target_bir_lowering=False, debug=False,
                   num_devices=N_CORES)
    if swi:
        xst_ap = nc.dram_tensor(
            "xst", [PAIRS, DRCH, P, NSC, 2 * P], FP8, kind="ExternalInput").ap()
    else:
        xst_ap = nc.dram_tensor(
            "xst", [PAIRS, DRCH, P, 2, T], FP8, kind="ExternalInput").ap()
    xpt_ap = nc.dram_tensor(
        "xpt", [PAIRS, DRCH, P, 2, T], FP8, kind="ExternalInput").ap()
    xv8_ap = nc.dram_tensor(
        "xv8", [PAIRS, NPC, P, 2, HD], FP8, kind="ExternalInput").ap()
    if tril8:
        tril8_ap = nc.dram_tensor("tril8", [2, P, 2, TB], FP8,
                                  kind="ExternalInput").ap()
    else:
        xv16_ap = nc.dram_tensor(
            "xv16", [PAIRS, NSC, P, HD], F16, kind="ExternalInput").ap()
        tril_ap = nc.dram_tensor("tril", [P, P], F16, kind="ExternalInput").ap()
    cvec_ap = nc.dram_tensor("cvec", [PAIRS, P, 1], F32, kind="ExternalInput").ap()
    out_ap = nc.dram_tensor("out", [PAIRS, T, HD], F16, kind="ExternalOutput").ap()

    with tile.TileContext(nc) as tc:
        with (
            tc.tile_pool(name="xst",
                         bufs=(2 * DRCH if swi else 2 * NTB * DRCH)) as xst_pool,
            tc.tile_pool(name="xpt", bufs=2 * NTB * DRCH) as xpt_pool,
            tc.tile_pool(name="dev", bufs=20) as dev_pool,
            tc.tile_pool(name="xv8", bufs=2 * NPC + 2) as xv8_pool,
            tc.tile_pool(name="xv16", bufs=(2 if tril8 else 2 * NSC + 2)) as xv16_pool,
            tc.tile_pool(name="tril", bufs=2) as tril_pool,
            tc.tile_pool(name="atmp", bufs=4) as a_pool,
            tc.tile_pool(name="osb", bufs=4) as osb_pool,
            tc.tile_pool(name="cvec", bufs=PAIRS) as c_pool,
            tc.tile_pool(name="psum_s", bufs=4, space="PSUM") as ps_pool,
            tc.tile_pool(name="psum_o", bufs=4, space="PSUM") as po_pool,
        ):
            if tril8:
                tril8_t = []
                for par in range(2):
                    tt = tril_pool.tile([P, 2, TB], FP8, name=f"tril8_{par}",
                                        tag="tril8")
                    nc.sync.dma_start(tt[:], tril8_ap[par])
                    tril8_t.append(tt)
                tril_t = None
            else:
                tril_t = tril_pool.tile([P, P], F16)
                nc.sync.dma_start(tril_t[:], tril_ap)
                tril8_t = None

            def load_pair(bh):
                cvec_t = c_pool.tile([P, 1], F32, name=f"cv_{bh}", tag="cv")
                nc.sync.dma_start(cvec_t[:], cvec_ap[bh])
                if swi:
                    xst_t = []
                    for k in range(DRCH):
                        t1 = xst_pool.tile([P, NSC, 2 * P], FP8,
                                           name=f"xst_{bh}_{k}", tag="xst")
                        nc.sync.dma_start(t1[:], xst_ap[bh, k])
                        xst_t.append(t1)
                else:
                    xst_t = [[None] * DRCH for _ in range(NTB)]
                xpt_t = [[None] * DRCH for _ in range(NTB)]
                for q in range(NTB):
                    for k in range(DRCH):
                        if not swi:
                            t1 = xst_pool.tile([P, 2, TB], FP8,
                                               name=f"xst_{bh}_{q}_{k}", tag="xst")
                            nc.sync.dma_start(
                                t1[:], xst_ap[bh, k, :, :, q * TB:(q + 1) * TB])
                            xst_t[q][k] = t1
                        t2 = xpt_pool.tile([P, 2, TB], FP8,
                                           name=f"xpt_{bh}_{q}_{k}", tag="xpt")
                        nc.sync.dma_start(
                            t2[:], xpt_ap[bh, k, :, :, q * TB:(q + 1) * TB])
                        xpt_t[q][k] = t2
                xv8_t = []
                for c2 in range(NPC):
                    t4 = xv8_pool.tile([P, 2, HD], FP8,
                                       name=f"xv8_{bh}_{c2}", tag="xv8")
                    nc.gpsimd.dma_start(t4[:], xv8_ap[bh, c2])
                    xv8_t.append(t4)
                xv16_t = None
                if not tril8:
                    xv16_t = []
                    for c in range(NSC):
                        t3 = xv16_pool.tile([P, HD], F16,
                                            name=f"xv16_{bh}_{c}", tag="xv16")
                        nc.gpsimd.dma_start(t3[:], xv16_ap[bh, c])
                        xv16_t.append(t3)
                return cvec_t, xst_t, xpt_t, xv8_t, xv16_t

            def emit_pair(bh, cvec_t, xst_t, xpt_t, xv8_t, xv16_t):
                dev_dr = [[None] * (2 * (j + 1)) for j in range(NTB)]

                def scores(j):
                    t0 = TB * j
                    nsc = (TB // P) * (j + 1)
                    for c in range(nsc):
                        c2, i2 = divmod(c, 2)
                        if i2 == 0:
                            dev_dr[j][c2] = dev_pool.tile(
                                [P, 2, TB], FP8, name=f"dev_{bh}_{j}_{c2}",
                                tag="dev")
                        dslice = dev_dr[j][c2][:, i2, :]
                        qc, rc = divmod(c, TB // P)
                        ps = ps_pool.tile([P, TB], F32)
                        for k in range(DRCH):
                            if swi:
                                lhsT = xst_t[k][:, c, :]
                                pm = mybir.MatmulPerfMode.DoubleRowSwInterleave
                            else:
                                lhsT = xst_t[qc][k][:, :, rc * P:(rc + 1) * P]
                                pm = mybir.MatmulPerfMode.DoubleRow
                            nc.tensor.matmul(
                                ps[:], lhsT, xpt_t[j][k][:],
                                start=(k == 0), stop=(k == DRCH - 1),
                                perf_mode=pm,
                            )
                        if sigmoid:
                            a = a_pool.tile([P, TB], F16)
                            nc.scalar.activation(
                                a[:], ps[:],
                                mybir.ActivationFunctionType.Sigmoid,
                                scale=cvec_t[:],
                            )
                            nc.vector.tensor_scalar(
                                dslice, a[:], SDEV, SDEV / 2,
                                op0=mybir.AluOpType.mult,
                                op1=mybir.AluOpType.subtract,
                            )
                        else:
                            # dev = SDEV*(sigmoid(c*raw)-1/2) ~= (SDEV*c/4)*raw
                            # (|c*raw| < 7e-3 for this model's statistics; the
                            #  cubic error ~1e-12 is far below fp8 resolution)
                            nc.scalar.activation(
                                dslice, ps[:],
                                mybir.ActivationFunctionType.Copy,
                                scale=cvec_t[:],
                            )
                        if c * P >= t0:  # diagonal/above: zero where t < s
                            nc.gpsimd.affine_select(
                                out=dslice, in_=dslice,
                                compare_op=mybir.AluOpType.is_ge,
                                fill=0.0,
                                base=t0 - c * P,
                                pattern=[[1, TB]],
                                channel_multiplier=-1,
                            )

                def av(j):
                    for il in range(4):
                        i = 4 * j + il
                        toff = il * P
                        osb = osb_pool.tile([P, HD], F16,
                                            name=f"osb_{bh}_{i}", tag="osb")
                        po = [po_pool.tile([P, TB], F32,
                                           name=f"po_{bh}_{i}_{h}", tag="po")
                              for h in range(2)]
                        # one LDWEIGHTS per stationary: both d-halves stream
                        # against the same weights back-to-back
                        if tril8:
                            for h in range(2):
                                nc.tensor.matmul(
                                    po[h][:], tril8_t[i % 2][:, :, 0:P],
                                    xv8_t[i // 2][:, :, h * TB:(h + 1) * TB],
                                    start=True, stop=False,
                                    perf_mode=mybir.MatmulPerfMode.DoubleRow,
                                )
                        else:
                            for h in range(2):
                                nc.tensor.matmul(
                                    po[h][:], tril_t[:],
                                    xv16_t[i][:, h * TB:(h + 1) * TB],
                                    start=True, stop=False,
                                )
                        nmax = i // 2
                        for c2 in range(nmax + 1):
                            for h in range(2):
                                nc.tensor.matmul(
                                    po[h][:],
                                    dev_dr[j][c2][:, :, toff:toff + P],
                                    xv8_t[c2][:, :, h * TB:(h + 1) * TB],
                                    start=False, stop=(c2 == nmax),
                                    perf_mode=mybir.MatmulPerfMode.DoubleRow,
                                )
                        for h in range(2):
                            nc.vector.tensor_copy(
                                osb[:, h * TB:(h + 1) * TB], po[h][:])
                        if not no_out:
                            nc.scalar.dma_start(
                                out_ap[bh, i * P:(i + 1) * P, :], osb[:])

                if no_scores:
                    for j in range(NTB):
                        for c2 in range(2 * (j + 1)):
                            dev_dr[j][c2] = dev_pool.tile(
                                [P, 2, TB], FP8, name=f"dev_{bh}_{j}_{c2}",
                                tag="dev")
                            nc.gpsimd.memset(dev_dr[j][c2][:], 0.25)
                        av(j)
                elif no_av:
                    for j in range(NTB):
                        scores(j)
                        osb = osb_pool.tile([P, HD], F16,
                                            name=f"osbx_{bh}_{j}", tag="osb")
                        nc.vector.tensor_copy(
                            osb[:, 0:TB], dev_dr[j][2 * j + 1][:, 1, :])
                        nc.scalar.dma_start(
                            out_ap[bh, 4 * j * P:(4 * j + 1) * P, :], osb[:])
                else:
                    scores(0)
                    for j in range(1, NTB):
                        scores(j)
                        av(j - 1)
                    av(NTB - 1)

            loaded = {}
            for bh in [bh for _ in range(reps) for bh in range(PAIRS)]:
                if load_once:
                    if bh not in loaded:
                        loaded[bh] = load_pair(bh)
                    tiles = loaded[bh]
                else:
                    tiles = load_pair(bh)
                emit_pair(bh, *tiles)

    nc.compile()
    return nc


def get_program():
    global _program_cache
    if _program_cache is None:
        _program_cache = _build_program()
    return _program_cache


def _sign_pm1(w):
    s = np.sign(w)
    return np.where(s == 0, 1.0, s).astype(np.float32)


def make_in_maps(x, bv_q, bv_k, bv_v, swi=SCORES_SWI, tril8=TRIL8):
    import ml_dtypes
    F8 = ml_dtypes.float8_e4m3fn

    x = np.asarray(x, dtype=np.float32)
    bv_q = np.asarray(bv_q, dtype=np.float32)
    bv_k = np.asarray(bv_k, dtype=np.float32)
    bv_v = np.asarray(bv_v, dtype=np.float32)

    alpha_q = np.abs(bv_q).mean(axis=-1)
    alpha_k = np.abs(bv_k).mean(axis=-1)
    alpha_v = np.abs(bv_v).mean(axis=-1)
    sgn_qk = _sign_pm1(bv_q) * _sign_pm1(bv_k)    # [H, HD]
    v_bind = alpha_v[:, None] * _sign_pm1(bv_v)   # [H, HD]
    c = (4.0 * (HD ** -0.5)) * alpha_q * alpha_k  # [H]

    triu = np.triu(np.ones((P, P), np.float32))
    tril16 = (triu * (SNET / 2)).astype(np.float16)
    t8 = np.zeros((2, P, 2, TB), np.float32)
    t8[0, :, 0, 0:P] = triu * (SDEV / 2)
    t8[1, :, 1, 0:P] = triu * (SDEV / 2)
    tril8_arr = t8.astype(F8)

    xh = x.reshape(B, T, H, HD)
    in_maps = []
    prefs = []   # per (b,h): [NSC, HD] f32 exclusive block-prefix of xv sums
    for core in range(N_CORES):
        if swi:
            xst = np.empty((PAIRS, DRCH, P, NSC, 2 * P), F8)
        else:
            xst = np.empty((PAIRS, DRCH, P, 2, T), F8)
        xpt = np.empty((PAIRS, DRCH, P, 2, T), F8)
        xv16 = np.empty((PAIRS, NSC, P, HD), np.float16)
        xv8 = np.empty((PAIRS, NPC, P, 2, HD), F8)
        cvec = np.empty((PAIRS, P, 1), np.float32)
        for slot in range(PAIRS):
            bh = PAIRS * core + slot
            b, h = divmod(bh, H)
            xs = xh[b, :, h, :]                      # [T, HD] f32
            xsT = np.ascontiguousarray(xs.T)         # [HD, T]
            xss = xsT * sgn_qk[h][:, None]
            # DR layouts: d = 256r + 128i + p
            dr = xss.reshape(DRCH, 2, P, T)
            if swi:
                # per 128-s-block: A/B interleave, columns reversed
                a = dr.reshape(DRCH, 2, P, NSC, P)   # [r, i, p, rc, mm]
                a = a[..., ::-1]                     # reverse mm
                a = a.transpose(0, 2, 3, 4, 1)       # [r, p, rc, mm_r, i]
                xst[slot] = a.reshape(DRCH, P, NSC, 2 * P).astype(F8)
            else:
                xst[slot] = dr.transpose(0, 2, 1, 3).astype(F8)
            xpt[slot] = xsT.reshape(DRCH, 2, P, T).transpose(0, 2, 1, 3).astype(F8)
            xv = xs * v_bind[h][None, :]             # [T, HD]
            xv16[slot] = xv.reshape(NSC, P, HD).astype(np.float16)
            xv8[slot] = (SV * xv).reshape(NPC, 2, P, HD).transpose(
                0, 2, 1, 3).astype(F8)
            cvec[slot] = c[h] if SIGMOID else c[h] * SDEV / 4
            bsum = xv.reshape(NSC, P, HD).sum(axis=1)          # [NSC, HD]
            pref = np.cumsum(bsum, axis=0) - bsum              # exclusive
            prefs.append(pref.astype(np.float32))
        m = {"xst": xst, "xpt": xpt, "xv8": xv8, "cvec": cvec}
        if tril8:
            m["tril8"] = tril8_arr
        else:
            m["xv16"] = xv16
            m["tril"] = tril16
        in_maps.append(m)
    return in_maps, prefs


def assemble_output(results, prefs):
    out = np.empty((B, T, D), np.float32)
    oh = out.reshape(B, T, H, HD)
    for core in range(N_CORES):
        for slot in range(PAIRS):
            bh = PAIRS * core + slot
            b, h = divmod(bh, H)
            raw = results[core]["out"][slot].astype(np.float32) / SNET
            raw.reshape(NSC, P, HD)[:] += 0.5 * prefs[bh][:, None, :]
            oh[b, :, h, :] = raw
    return out


def kernel(x, bv_q, bv_k, bv_v):
    nc = get_program()
    in_maps, prefs = make_in_maps(x, bv_q, bv_k, bv_v)
    res = run_bass_kernel_spmd(nc, in_maps, list(range(N_CORES)))
    return assemble_output(res.results, prefs)


# revision 6
# speedup vs baseline: 1.3538x; 1.3538x over previous
"""Trainium2 Bass kernel v2 for nn_MultiHeadBindingAttention.

Math (per (b,h) pair; T=2048, HD=1024):
    z[t,s]   = c_h * raw[t,s],  raw = x @ (x*sgn_qk)^T   (symmetric)
    attn     = causal(sigmoid(z)) = 0.5*tril + dev,  dev = causal(sigmoid(z) - 0.5)
    out[t,:] = attn @ xv,  xv = x * v_bind

Device computes:  raw_out = SNET * ( dev @ xv  +  0.5 * triu_diagblock @ xv )
  - dev tiles in fp8e4 (scaled by SDEV), DoubleRow-packed; all AV matmuls
    (dev and the intra-128-block 0.5*triangle term) run fp8 DoubleRow with
    the same moving operand xv8 = SV*xv, so the PE never switches modes
    inside the AV phase.
  - the inter-block part of the 0.5-term (0.5 * sum_{s < block} xv[s,:]) is
    a per-block bias row the host adds in the epilogue.
  - scores matmuls in fp8 DoubleRowSwInterleave (software-interleaved
    stationary -> faster weight path).

Sharding: 16 (b,h) pairs data-parallel over 8 cores, 2 pairs/core.
"""

import numpy as np

import concourse.bacc as bacc
import concourse.tile as tile
from concourse import mybir
from concourse.bass_utils import run_bass_kernel_spmd

B, T, D = 4, 2048, 4096
H, HD = 4, 1024
N_CORES = 8
PAIRS = 2
P = 128
TB = 512
NTB = T // TB                  # 4 strips
NSC = T // P                   # 16 s-chunks
NPC = NSC // 2                 # 8 s-pair chunks (DoubleRow)
DRCH = HD // (2 * P)           # 4 score contraction chunks of 256

F32 = mybir.dt.float32
F16 = mybir.dt.float16
FP8 = mybir.dt.float8e4

SDEV = 128.0                   # dev tile scale
SV = 256.0                     # xv fp8 scale
SNET = SDEV * SV               # net output scale (= 32768)

SIGMOID = True                 # True: real sigmoid + DVE affine; False: linear Copy
SCORES_SWI = True              # SwInterleave stationary for scores
TRIL8 = True                   # fp8-DR intra-block triangle (else fp16 + xv16)
HOST_TRIL = True               # 0.5*cumsum(xv) bias added on host; device does dev@xv only

_program_cache = None


def _build_program(reps=1, no_av=False, no_scores=False,
                   sigmoid=SIGMOID, swi=SCORES_SWI, tril8=TRIL8,
                   load_once=False, no_out=False, host_tril=HOST_TRIL,
                   ps_bufs=4, po_bufs=4, a_bufs=4):
    nc = bacc.Bacc(trn_type="TRN2", target_bir_lowering=False, debug=False,
                   num_devices=N_CORES)
    if swi:
        xst_ap = nc.dram_tensor(
            "xst", [PAIRS, DRCH, P, NSC, 2 * P], FP8, kind="ExternalInput").ap()
    else:
        xst_ap = nc.dram_tensor(
            "xst", [PAIRS, DRCH, P, 2, T], FP8, kind="ExternalInput").ap()
    xpt_ap = nc.dram_tensor(
        "xpt", [PAIRS, DRCH, P, 2, T], FP8, kind="ExternalInput").ap()
    xv8_ap = nc.dram_tensor(
        "xv8", [PAIRS, NPC, P, 2, HD], FP8, kind="ExternalInput").ap()
    if host_tril:
        pass
    elif tril8:
        tril8_ap = nc.dram_tensor("tril8", [2, P, 2, TB], FP8,
                                  kind="ExternalInput").ap()
    else:
        xv16_ap = nc.dram_tensor(
            "xv16", [PAIRS, NSC, P, HD], F16, kind="ExternalInput").ap()
        tril_ap = nc.dram_tensor("tril", [P, P], F16, kind="ExternalInput").ap()
    cvec_ap = nc.dram_tensor("cvec", [PAIRS, P, 1], F32, kind="ExternalInput").ap()
    out_ap = nc.dram_tensor("out", [PAIRS, T, HD], F16, kind="ExternalOutput").ap()

    with tile.TileContext(nc) as tc:
        with (
            tc.tile_pool(name="xst",
                         bufs=(2 * DRCH if swi else 2 * NTB * DRCH)) as xst_pool,
            tc.tile_pool(name="xpt", bufs=2 * NTB * DRCH) as xpt_pool,
            tc.tile_pool(name="dev", bufs=20) as dev_pool,
            tc.tile_pool(name="xv8", bufs=2 * NPC + 2) as xv8_pool,
            tc.tile_pool(name="xv16", bufs=(2 if tril8 else 2 * NSC + 2)) as xv16_pool,
            tc.tile_pool(name="tril", bufs=2) as tril_pool,
            tc.tile_pool(name="atmp", bufs=a_bufs) as a_pool,
            tc.tile_pool(name="osb", bufs=4) as osb_pool,
            tc.tile_pool(name="cvec", bufs=PAIRS) as c_pool,
            tc.tile_pool(name="psum_s", bufs=ps_bufs, space="PSUM") as ps_pool,
            tc.tile_pool(name="psum_o", bufs=po_bufs, space="PSUM") as po_pool,
        ):
            if host_tril:
                tril8_t = tril_t = None
            elif tril8:
                tril8_t = []
                for par in range(2):
                    tt = tril_pool.tile([P, 2, TB], FP8, name=f"tril8_{par}",
                                        tag="tril8")
                    nc.sync.dma_start(tt[:], tril8_ap[par])
                    tril8_t.append(tt)
                tril_t = None
            else:
                tril_t = tril_pool.tile([P, P], F16)
                nc.sync.dma_start(tril_t[:], tril_ap)
                tril8_t = None

            def load_pair(bh):
                cvec_t = c_pool.tile([P, 1], F32, name=f"cv_{bh}", tag="cv")
                nc.sync.dma_start(cvec_t[:], cvec_ap[bh])
                if swi:
                    xst_t = []
                    for k in range(DRCH):
                        t1 = xst_pool.tile([P, NSC, 2 * P], FP8,
                                           name=f"xst_{bh}_{k}", tag="xst")
                        nc.sync.dma_start(t1[:], xst_ap[bh, k])
                        xst_t.append(t1)
                else:
                    xst_t = [[None] * DRCH for _ in range(NTB)]
                xpt_t = [[None] * DRCH for _ in range(NTB)]
                for q in range(NTB):
                    for k in range(DRCH):
                        if not swi:
                            t1 = xst_pool.tile([P, 2, TB], FP8,
                                               name=f"xst_{bh}_{q}_{k}", tag="xst")
                            nc.sync.dma_start(
                                t1[:], xst_ap[bh, k, :, :, q * TB:(q + 1) * TB])
                            xst_t[q][k] = t1
                        t2 = xpt_pool.tile([P, 2, TB], FP8,
                                           name=f"xpt_{bh}_{q}_{k}", tag="xpt")
                        nc.sync.dma_start(
                            t2[:], xpt_ap[bh, k, :, :, q * TB:(q + 1) * TB])
                        xpt_t[q][k] = t2
                xv8_t = []
                for c2 in range(NPC):
                    t4 = xv8_pool.tile([P, 2, HD], FP8,
                                       name=f"xv8_{bh}_{c2}", tag="xv8")
                    nc.gpsimd.dma_start(t4[:], xv8_ap[bh, c2])
                    xv8_t.append(t4)
                xv16_t = None
                if not tril8:
                    xv16_t = []
                    for c in range(NSC):
                        t3 = xv16_pool.tile([P, HD], F16,
                                            name=f"xv16_{bh}_{c}", tag="xv16")
                        nc.gpsimd.dma_start(t3[:], xv16_ap[bh, c])
                        xv16_t.append(t3)
                return cvec_t, xst_t, xpt_t, xv8_t, xv16_t

            def emit_pair(bh, cvec_t, xst_t, xpt_t, xv8_t, xv16_t):
                dev_dr = [[None] * (2 * (j + 1)) for j in range(NTB)]

                def scores(j):
                    t0 = TB * j
                    nsc = (TB // P) * (j + 1)
                    for c in range(nsc):
                        c2, i2 = divmod(c, 2)
                        if i2 == 0:
                            dev_dr[j][c2] = dev_pool.tile(
                                [P, 2, TB], FP8, name=f"dev_{bh}_{j}_{c2}",
                                tag="dev")
                        # diagonal trim: tile (j, c) only needs t >= toff.
                        # For odd chunks the 128 cols just below the trim are
                        # consumed (as zeros) by the t-block one below the
                        # diagonal, so memset them instead of computing.
                        il_c = c - 4 * j
                        toff = max(0, il_c) * P
                        if toff and i2 == 1:
                            nc.vector.memset(
                                dev_dr[j][c2][:, 1, toff - P:toff], 0.0)
                        nt = TB - toff
                        dslice = dev_dr[j][c2][:, i2, toff:TB]
                        qc, rc = divmod(c, TB // P)
                        ps = ps_pool.tile([P, nt], F32, name=f"ps_{bh}_{j}_{c}",
                                          tag="ps")
                        for k in range(DRCH):
                            if swi:
                                lhsT = xst_t[k][:, c, :]
                                pm = mybir.MatmulPerfMode.DoubleRowSwInterleave
                            else:
                                lhsT = xst_t[qc][k][:, :, rc * P:(rc + 1) * P]
                                pm = mybir.MatmulPerfMode.DoubleRow
                            nc.tensor.matmul(
                                ps[:], lhsT, xpt_t[j][k][:, :, toff:TB],
                                start=(k == 0), stop=(k == DRCH - 1),
                                perf_mode=pm,
                            )
                        if sigmoid:
                            a = a_pool.tile([P, nt], F16, name=f"a_{bh}_{j}_{c}",
                                            tag="a")
                            nc.scalar.activation(
                                a[:], ps[:],
                                mybir.ActivationFunctionType.Sigmoid,
                                scale=cvec_t[:],
                            )
                            nc.vector.tensor_scalar(
                                dslice, a[:], SDEV, SDEV / 2,
                                op0=mybir.AluOpType.mult,
                                op1=mybir.AluOpType.subtract,
                            )
                        else:
                            # dev = SDEV*(sigmoid(c*raw)-1/2) ~= (SDEV*c/4)*raw
                            # (|c*raw| < 7e-3 for this model's statistics; the
                            #  cubic error ~1e-12 is far below fp8 resolution)
                            nc.scalar.activation(
                                dslice, ps[:],
                                mybir.ActivationFunctionType.Copy,
                                scale=cvec_t[:],
                            )
                        if c * P >= t0:  # diagonal/above: zero where t < s
                            nc.gpsimd.affine_select(
                                out=dslice, in_=dslice,
                                compare_op=mybir.AluOpType.is_ge,
                                fill=0.0,
                                base=t0 + toff - c * P,
                                pattern=[[1, nt]],
                                channel_multiplier=-1,
                            )

                def av(j):
                    for il in range(4):
                        i = 4 * j + il
                        toff = il * P
                        osb = osb_pool.tile([P, HD], F16,
                                            name=f"osb_{bh}_{i}", tag="osb")
                        po = [po_pool.tile([P, TB], F32,
                                           name=f"po_{bh}_{i}_{h}", tag="po")
                              for h in range(2)]
                        # one LDWEIGHTS per stationary: both d-halves stream
                        # against the same weights back-to-back
                        if not host_tril:
                            if tril8:
                                for h in range(2):
                                    nc.tensor.matmul(
                                        po[h][:], tril8_t[i % 2][:, :, 0:P],
                                        xv8_t[i // 2][:, :, h * TB:(h + 1) * TB],
                                        start=True, stop=False,
                                        perf_mode=mybir.MatmulPerfMode.DoubleRow,
                                    )
                            else:
                                for h in range(2):
                                    nc.tensor.matmul(
                                        po[h][:], tril_t[:],
                                        xv16_t[i][:, h * TB:(h + 1) * TB],
                                        start=True, stop=False,
                                    )
                        nmax = i // 2
                        for c2 in range(nmax + 1):
                            for h in range(2):
                                nc.tensor.matmul(
                                    po[h][:],
                                    dev_dr[j][c2][:, :, toff:toff + P],
                                    xv8_t[c2][:, :, h * TB:(h + 1) * TB],
                                    start=(host_tril and c2 == 0),
                                    stop=(c2 == nmax),
                                    perf_mode=mybir.MatmulPerfMode.DoubleRow,
                                )
                        for h in range(2):
                            nc.vector.tensor_copy(
                                osb[:, h * TB:(h + 1) * TB], po[h][:])
                        if not no_out:
                            nc.scalar.dma_start(
                                out_ap[bh, i * P:(i + 1) * P, :], osb[:])

                if no_scores:
                    for j in range(NTB):
                        for c2 in range(2 * (j + 1)):
                            dev_dr[j][c2] = dev_pool.tile(
                                [P, 2, TB], FP8, name=f"dev_{bh}_{j}_{c2}",
                                tag="dev")
                            nc.gpsimd.memset(dev_dr[j][c2][:], 0.25)
                        av(j)
                elif no_av:
                    for j in range(NTB):
                        scores(j)
                        osb = osb_pool.tile([P, HD], F16,
                                            name=f"osbx_{bh}_{j}", tag="osb")
                        nc.vector.tensor_copy(
                            osb[:, 0:TB], dev_dr[j][2 * j + 1][:, 1, :])
                        nc.scalar.dma_start(
                            out_ap[bh, 4 * j * P:(4 * j + 1) * P, :], osb[:])
                else:
                    scores(0)
                    for j in range(1, NTB):
                        scores(j)
                        av(j - 1)
                    av(NTB - 1)

            loaded = {}
            for bh in [bh for _ in range(reps) for bh in range(PAIRS)]:
                if load_once:
                    if bh not in loaded:
                        loaded[bh] = load_pair(bh)
                    tiles = loaded[bh]
                else:
                    tiles = load_pair(bh)
                emit_pair(bh, *tiles)

    nc.compile()
    return nc


def get_program():
    global _program_cache
    if _program_cache is None:
        _program_cache = _build_program()
    return _program_cache


def _sign_pm1(w):
    s = np.sign(w)
    return np.where(s == 0, 1.0, s).astype(np.float32)


def make_in_maps(x, bv_q, bv_k, bv_v, swi=SCORES_SWI, tril8=TRIL8,
                 host_tril=HOST_TRIL):
    import ml_dtypes
    F8 = ml_dtypes.float8_e4m3fn

    x = np.asarray(x, dtype=np.float32)
    bv_q = np.asarray(bv_q, dtype=np.float32)
    bv_k = np.asarray(bv_k, dtype=np.float32)
    bv_v = np.asarray(bv_v, dtype=np.float32)

    alpha_q = np.abs(bv_q).mean(axis=-1)
    alpha_k = np.abs(bv_k).mean(axis=-1)
    alpha_v = np.abs(bv_v).mean(axis=-1)
    sgn_qk = _sign_pm1(bv_q) * _sign_pm1(bv_k)    # [H, HD]
    v_bind = alpha_v[:, None] * _sign_pm1(bv_v)   # [H, HD]
    c = (4.0 * (HD ** -0.5)) * alpha_q * alpha_k  # [H]

    triu = np.triu(np.ones((P, P), np.float32))
    tril16 = (triu * (SNET / 2)).astype(np.float16)
    t8 = np.zeros((2, P, 2, TB), np.float32)
    t8[0, :, 0, 0:P] = triu * (SDEV / 2)
    t8[1, :, 1, 0:P] = triu * (SDEV / 2)
    tril8_arr = t8.astype(F8)

    xh = x.reshape(B, T, H, HD)
    in_maps = []
    prefs = []   # per (b,h): [NSC, HD] f32 exclusive block-prefix of xv sums
    for core in range(N_CORES):
        if swi:
            xst = np.empty((PAIRS, DRCH, P, NSC, 2 * P), F8)
        else:
            xst = np.empty((PAIRS, DRCH, P, 2, T), F8)
        xpt = np.empty((PAIRS, DRCH, P, 2, T), F8)
        xv16 = np.empty((PAIRS, NSC, P, HD), np.float16)
        xv8 = np.empty((PAIRS, NPC, P, 2, HD), F8)
        cvec = np.empty((PAIRS, P, 1), np.float32)
        for slot in range(PAIRS):
            bh = PAIRS * core + slot
            b, h = divmod(bh, H)
            xs = xh[b, :, h, :]                      # [T, HD] f32
            xsT = np.ascontiguousarray(xs.T)         # [HD, T]
            xss = xsT * sgn_qk[h][:, None]
            # DR layouts: d = 256r + 128i + p
            dr = xss.reshape(DRCH, 2, P, T)
            if swi:
                # per 128-s-block: A/B interleave, columns reversed
                a = dr.reshape(DRCH, 2, P, NSC, P)   # [r, i, p, rc, mm]
                a = a[..., ::-1]                     # reverse mm
                a = a.transpose(0, 2, 3, 4, 1)       # [r, p, rc, mm_r, i]
                xst[slot] = a.reshape(DRCH, P, NSC, 2 * P).astype(F8)
            else:
                xst[slot] = dr.transpose(0, 2, 1, 3).astype(F8)
            xpt[slot] = xsT.reshape(DRCH, 2, P, T).transpose(0, 2, 1, 3).astype(F8)
            xv = xs * v_bind[h][None, :]             # [T, HD]
            xv16[slot] = xv.reshape(NSC, P, HD).astype(np.float16)
            xv8[slot] = (SV * xv).reshape(NPC, 2, P, HD).transpose(
                0, 2, 1, 3).astype(F8)
            cvec[slot] = c[h] if SIGMOID else c[h] * SDEV / 4
            if host_tril:
                # full 0.5 * inclusive cumsum bias, added in the epilogue
                prefs.append(0.5 * np.cumsum(xv, axis=0, dtype=np.float32))
            else:
                bsum = xv.reshape(NSC, P, HD).sum(axis=1)      # [NSC, HD]
                pref = np.cumsum(bsum, axis=0) - bsum          # exclusive
                prefs.append(pref.astype(np.float32))
        m = {"xst": xst, "xpt": xpt, "xv8": xv8, "cvec": cvec}
        if not host_tril:
            if tril8:
                m["tril8"] = tril8_arr
            else:
                m["xv16"] = xv16
                m["tril"] = tril16
        in_maps.append(m)
    return in_maps, prefs


def assemble_output(results, prefs):
    out = np.empty((B, T, D), np.float32)
    oh = out.reshape(B, T, H, HD)
    for core in range(N_CORES):
        for slot in range(PAIRS):
            bh = PAIRS * core + slot
            b, h = divmod(bh, H)
            raw = results[core]["out"][slot].astype(np.float32) / SNET
            if prefs[bh].shape == (T, HD):
                raw += prefs[bh]
            else:
                raw.reshape(NSC, P, HD)[:] += 0.5 * prefs[bh][:, None, :]
            oh[b, :, h, :] = raw
    return out


def kernel(x, bv_q, bv_k, bv_v):
    nc = get_program()
    in_maps, prefs = make_in_maps(x, bv_q, bv_k, bv_v)
    res = run_bass_kernel_spmd(nc, in_maps, list(range(N_CORES)))
    return assemble_output(res.results, prefs)


# revision 7
# speedup vs baseline: 1.4062x; 1.0387x over previous
"""Trainium2 Bass kernel v2 for nn_MultiHeadBindingAttention.

Math (per (b,h) pair; T=2048, HD=1024):
    z[t,s]   = c_h * raw[t,s],  raw = x @ (x*sgn_qk)^T   (symmetric)
    attn     = causal(sigmoid(z)) = 0.5*tril + dev,  dev = causal(sigmoid(z) - 0.5)
    out[t,:] = attn @ xv,  xv = x * v_bind

Device computes:  raw_out = SNET * ( dev @ xv  +  0.5 * triu_diagblock @ xv )
  - dev tiles in fp8e4 (scaled by SDEV), DoubleRow-packed; all AV matmuls
    (dev and the intra-128-block 0.5*triangle term) run fp8 DoubleRow with
    the same moving operand xv8 = SV*xv, so the PE never switches modes
    inside the AV phase.
  - the inter-block part of the 0.5-term (0.5 * sum_{s < block} xv[s,:]) is
    a per-block bias row the host adds in the epilogue.
  - scores matmuls in fp8 DoubleRowSwInterleave (software-interleaved
    stationary -> faster weight path).

Sharding: 16 (b,h) pairs data-parallel over 8 cores, 2 pairs/core.
"""

import numpy as np

import concourse.bacc as bacc
import concourse.tile as tile
from concourse import mybir
from concourse.bass_utils import run_bass_kernel_spmd

B, T, D = 4, 2048, 4096
H, HD = 4, 1024
N_CORES = 8
PAIRS = 2
P = 128
TB = 512
NTB = T // TB                  # 4 strips
NSC = T // P                   # 16 s-chunks
NPC = NSC // 2                 # 8 s-pair chunks (DoubleRow)
DRCH = HD // (2 * P)           # 4 score contraction chunks of 256

F32 = mybir.dt.float32
F16 = mybir.dt.float16
FP8 = mybir.dt.float8e4

SDEV = 128.0                   # dev tile scale
SV = 256.0                     # xv fp8 scale
SNET = SDEV * SV               # net output scale (= 32768)

SIGMOID = True                 # True: real sigmoid + DVE affine; False: linear Copy
SCORES_SWI = True              # SwInterleave stationary for scores
TRIL8 = True                   # fp8-DR intra-block triangle (else fp16 + xv16)
HOST_TRIL = True               # 0.5*cumsum(xv) bias added on host; device does dev@xv only

_program_cache = None


def _build_program(reps=1, no_av=False, no_scores=False,
                   sigmoid=SIGMOID, swi=SCORES_SWI, tril8=TRIL8,
                   load_once=False, no_out=False, host_tril=HOST_TRIL,
                   ps_bufs=5, po_bufs=3, a_bufs=4):
    nc = bacc.Bacc(trn_type="TRN2", target_bir_lowering=False, debug=False,
                   num_devices=N_CORES)
    if swi:
        xst_ap = nc.dram_tensor(
            "xst", [PAIRS, DRCH, P, NSC, 2 * P], FP8, kind="ExternalInput").ap()
    else:
        xst_ap = nc.dram_tensor(
            "xst", [PAIRS, DRCH, P, 2, T], FP8, kind="ExternalInput").ap()
    xpt_ap = nc.dram_tensor(
        "xpt", [PAIRS, DRCH, P, 2, T], FP8, kind="ExternalInput").ap()
    xv8_ap = nc.dram_tensor(
        "xv8", [PAIRS, NPC, P, 2, HD], FP8, kind="ExternalInput").ap()
    if host_tril:
        pass
    elif tril8:
        tril8_ap = nc.dram_tensor("tril8", [2, P, 2, TB], FP8,
                                  kind="ExternalInput").ap()
    else:
        xv16_ap = nc.dram_tensor(
            "xv16", [PAIRS, NSC, P, HD], F16, kind="ExternalInput").ap()
        tril_ap = nc.dram_tensor("tril", [P, P], F16, kind="ExternalInput").ap()
    cvec_ap = nc.dram_tensor("cvec", [PAIRS, P, 1], F32, kind="ExternalInput").ap()
    out_ap = nc.dram_tensor("out", [PAIRS, T, HD], F16, kind="ExternalOutput").ap()

    with tile.TileContext(nc) as tc:
        with (
            tc.tile_pool(name="xst",
                         bufs=(2 * DRCH if swi else 2 * NTB * DRCH)) as xst_pool,
            tc.tile_pool(name="xpt", bufs=2 * NTB * DRCH) as xpt_pool,
            tc.tile_pool(name="dev", bufs=20) as dev_pool,
            tc.tile_pool(name="xv8", bufs=2 * NPC + 2) as xv8_pool,
            tc.tile_pool(name="xv16", bufs=(2 if tril8 else 2 * NSC + 2)) as xv16_pool,
            tc.tile_pool(name="tril", bufs=2) as tril_pool,
            tc.tile_pool(name="atmp", bufs=a_bufs) as a_pool,
            tc.tile_pool(name="osb", bufs=4) as osb_pool,
            tc.tile_pool(name="cvec", bufs=PAIRS) as c_pool,
            tc.tile_pool(name="psum_s", bufs=ps_bufs, space="PSUM") as ps_pool,
            tc.tile_pool(name="psum_o", bufs=po_bufs, space="PSUM") as po_pool,
        ):
            if host_tril:
                tril8_t = tril_t = None
            elif tril8:
                tril8_t = []
                for par in range(2):
                    tt = tril_pool.tile([P, 2, TB], FP8, name=f"tril8_{par}",
                                        tag="tril8")
                    nc.sync.dma_start(tt[:], tril8_ap[par])
                    tril8_t.append(tt)
                tril_t = None
            else:
                tril_t = tril_pool.tile([P, P], F16)
                nc.sync.dma_start(tril_t[:], tril_ap)
                tril8_t = None

            def load_pair(bh):
                cvec_t = c_pool.tile([P, 1], F32, name=f"cv_{bh}", tag="cv")
                nc.sync.dma_start(cvec_t[:], cvec_ap[bh])
                if swi:
                    xst_t = []
                    for k in range(DRCH):
                        t1 = xst_pool.tile([P, NSC, 2 * P], FP8,
                                           name=f"xst_{bh}_{k}", tag="xst")
                        nc.sync.dma_start(t1[:], xst_ap[bh, k])
                        xst_t.append(t1)
                else:
                    xst_t = [[None] * DRCH for _ in range(NTB)]
                xpt_t = [[None] * DRCH for _ in range(NTB)]
                for q in range(NTB):
                    for k in range(DRCH):
                        if not swi:
                            t1 = xst_pool.tile([P, 2, TB], FP8,
                                               name=f"xst_{bh}_{q}_{k}", tag="xst")
                            nc.sync.dma_start(
                                t1[:], xst_ap[bh, k, :, :, q * TB:(q + 1) * TB])
                            xst_t[q][k] = t1
                        t2 = xpt_pool.tile([P, 2, TB], FP8,
                                           name=f"xpt_{bh}_{q}_{k}", tag="xpt")
                        nc.sync.dma_start(
                            t2[:], xpt_ap[bh, k, :, :, q * TB:(q + 1) * TB])
                        xpt_t[q][k] = t2
                xv8_t = []
                for c2 in range(NPC):
                    t4 = xv8_pool.tile([P, 2, HD], FP8,
                                       name=f"xv8_{bh}_{c2}", tag="xv8")
                    nc.gpsimd.dma_start(t4[:], xv8_ap[bh, c2])
                    xv8_t.append(t4)
                xv16_t = None
                if not tril8:
                    xv16_t = []
                    for c in range(NSC):
                        t3 = xv16_pool.tile([P, HD], F16,
                                            name=f"xv16_{bh}_{c}", tag="xv16")
                        nc.gpsimd.dma_start(t3[:], xv16_ap[bh, c])
                        xv16_t.append(t3)
                return cvec_t, xst_t, xpt_t, xv8_t, xv16_t

            def emit_pair(bh, cvec_t, xst_t, xpt_t, xv8_t, xv16_t):
                dev_dr = [[None] * (2 * (j + 1)) for j in range(NTB)]

                def scores(j):
                    t0 = TB * j
                    nsc = (TB // P) * (j + 1)
                    for c in range(nsc):
                        c2, i2 = divmod(c, 2)
                        if i2 == 0:
                            dev_dr[j][c2] = dev_pool.tile(
                                [P, 2, TB], FP8, name=f"dev_{bh}_{j}_{c2}",
                                tag="dev")
                        # diagonal trim: tile (j, c) only needs t >= toff.
                        # For odd chunks the 128 cols just below the trim are
                        # consumed (as zeros) by the t-block one below the
                        # diagonal, so memset them instead of computing.
                        il_c = c - 4 * j
                        toff = max(0, il_c) * P
                        if toff and i2 == 1:
                            nc.vector.memset(
                                dev_dr[j][c2][:, 1, toff - P:toff], 0.0)
                        nt = TB - toff
                        dslice = dev_dr[j][c2][:, i2, toff:TB]
                        qc, rc = divmod(c, TB // P)
                        ps = ps_pool.tile([P, nt], F32, name=f"ps_{bh}_{j}_{c}",
                                          tag="ps")
                        for k in range(DRCH):
                            if swi:
                                lhsT = xst_t[k][:, c, :]
                                pm = mybir.MatmulPerfMode.DoubleRowSwInterleave
                            else:
                                lhsT = xst_t[qc][k][:, :, rc * P:(rc + 1) * P]
                                pm = mybir.MatmulPerfMode.DoubleRow
                            nc.tensor.matmul(
                                ps[:], lhsT, xpt_t[j][k][:, :, toff:TB],
                                start=(k == 0), stop=(k == DRCH - 1),
                                perf_mode=pm,
                            )
                        if sigmoid:
                            a = a_pool.tile([P, nt], F16, name=f"a_{bh}_{j}_{c}",
                                            tag="a")
                            nc.scalar.activation(
                                a[:], ps[:],
                                mybir.ActivationFunctionType.Sigmoid,
                                scale=cvec_t[:],
                            )
                            nc.vector.tensor_scalar(
                                dslice, a[:], SDEV, SDEV / 2,
                                op0=mybir.AluOpType.mult,
                                op1=mybir.AluOpType.subtract,
                            )
                        else:
                            # dev = SDEV*(sigmoid(c*raw)-1/2) ~= (SDEV*c/4)*raw
                            # (|c*raw| < 7e-3 for this model's statistics; the
                            #  cubic error ~1e-12 is far below fp8 resolution)
                            nc.scalar.activation(
                                dslice, ps[:],
                                mybir.ActivationFunctionType.Copy,
                                scale=cvec_t[:],
                            )
                        if c * P >= t0:  # diagonal/above: zero where t < s
                            nc.gpsimd.affine_select(
                                out=dslice, in_=dslice,
                                compare_op=mybir.AluOpType.is_ge,
                                fill=0.0,
                                base=t0 + toff - c * P,
                                pattern=[[1, nt]],
                                channel_multiplier=-1,
                            )

                def av(j):
                    for il in range(4):
                        i = 4 * j + il
                        toff = il * P
                        osb = osb_pool.tile([P, HD], F16,
                                            name=f"osb_{bh}_{i}", tag="osb")
                        po = [po_pool.tile([P, TB], F32,
                                           name=f"po_{bh}_{i}_{h}", tag="po")
                              for h in range(2)]
                        # one LDWEIGHTS per stationary: both d-halves stream
                        # against the same weights back-to-back
                        if not host_tril:
                            if tril8:
                                for h in range(2):
                                    nc.tensor.matmul(
                                        po[h][:], tril8_t[i % 2][:, :, 0:P],
                                        xv8_t[i // 2][:, :, h * TB:(h + 1) * TB],
                                        start=True, stop=False,
                                        perf_mode=mybir.MatmulPerfMode.DoubleRow,
                                    )
                            else:
                                for h in range(2):
                                    nc.tensor.matmul(
                                        po[h][:], tril_t[:],
                                        xv16_t[i][:, h * TB:(h + 1) * TB],
                                        start=True, stop=False,
                                    )
                        nmax = i // 2
                        for c2 in range(nmax + 1):
                            for h in range(2):
                                nc.tensor.matmul(
                                    po[h][:],
                                    dev_dr[j][c2][:, :, toff:toff + P],
                                    xv8_t[c2][:, :, h * TB:(h + 1) * TB],
                                    start=(host_tril and c2 == 0),
                                    stop=(c2 == nmax),
                                    perf_mode=mybir.MatmulPerfMode.DoubleRow,
                                )
                        for h in range(2):
                            nc.vector.tensor_copy(
                                osb[:, h * TB:(h + 1) * TB], po[h][:])
                        if not no_out:
                            nc.scalar.dma_start(
                                out_ap[bh, i * P:(i + 1) * P, :], osb[:])

                if no_scores:
                    for j in range(NTB):
                        for c2 in range(2 * (j + 1)):
                            dev_dr[j][c2] = dev_pool.tile(
                                [P, 2, TB], FP8, name=f"dev_{bh}_{j}_{c2}",
                                tag="dev")
                            nc.gpsimd.memset(dev_dr[j][c2][:], 0.25)
                        av(j)
                elif no_av:
                    for j in range(NTB):
                        scores(j)
                        osb = osb_pool.tile([P, HD], F16,
                                            name=f"osbx_{bh}_{j}", tag="osb")
                        nc.vector.tensor_copy(
                            osb[:, 0:TB], dev_dr[j][2 * j + 1][:, 1, :])
                        nc.scalar.dma_start(
                            out_ap[bh, 4 * j * P:(4 * j + 1) * P, :], osb[:])
                else:
                    scores(0)
                    for j in range(1, NTB):
                        scores(j)
                        av(j - 1)
                    av(NTB - 1)

            loaded = {}
            for bh in [bh for _ in range(reps) for bh in range(PAIRS)]:
                if load_once:
                    if bh not in loaded:
                        loaded[bh] = load_pair(bh)
                    tiles = loaded[bh]
                else:
                    tiles = load_pair(bh)
                emit_pair(bh, *tiles)

    nc.compile()
    return nc


def get_program():
    global _program_cache
    if _program_cache is None:
        _program_cache = _build_program()
    return _program_cache


def _sign_pm1(w):
    s = np.sign(w)
    return np.where(s == 0, 1.0, s).astype(np.float32)


def make_in_maps(x, bv_q, bv_k, bv_v, swi=SCORES_SWI, tril8=TRIL8,
                 host_tril=HOST_TRIL):
    import ml_dtypes
    F8 = ml_dtypes.float8_e4m3fn

    x = np.asarray(x, dtype=np.float32)
    bv_q = np.asarray(bv_q, dtype=np.float32)
    bv_k = np.asarray(bv_k, dtype=np.float32)
    bv_v = np.asarray(bv_v, dtype=np.float32)

    alpha_q = np.abs(bv_q).mean(axis=-1)
    alpha_k = np.abs(bv_k).mean(axis=-1)
    alpha_v = np.abs(bv_v).mean(axis=-1)
    sgn_qk = _sign_pm1(bv_q) * _sign_pm1(bv_k)    # [H, HD]
    v_bind = alpha_v[:, None] * _sign_pm1(bv_v)   # [H, HD]
    c = (4.0 * (HD ** -0.5)) * alpha_q * alpha_k  # [H]

    triu = np.triu(np.ones((P, P), np.float32))
    tril16 = (triu * (SNET / 2)).astype(np.float16)
    t8 = np.zeros((2, P, 2, TB), np.float32)
    t8[0, :, 0, 0:P] = triu * (SDEV / 2)
    t8[1, :, 1, 0:P] = triu * (SDEV / 2)
    tril8_arr = t8.astype(F8)

    xh = x.reshape(B, T, H, HD)
    in_maps = []
    prefs = []   # per (b,h): [NSC, HD] f32 exclusive block-prefix of xv sums
    for core in range(N_CORES):
        if swi:
            xst = np.empty((PAIRS, DRCH, P, NSC, 2 * P), F8)
        else:
            xst = np.empty((PAIRS, DRCH, P, 2, T), F8)
        xpt = np.empty((PAIRS, DRCH, P, 2, T), F8)
        xv16 = np.empty((PAIRS, NSC, P, HD), np.float16)
        xv8 = np.empty((PAIRS, NPC, P, 2, HD), F8)
        cvec = np.empty((PAIRS, P, 1), np.float32)
        for slot in range(PAIRS):
            bh = PAIRS * core + slot
            b, h = divmod(bh, H)
            xs = xh[b, :, h, :]                      # [T, HD] f32
            xsT = np.ascontiguousarray(xs.T)         # [HD, T]
            xss = xsT * sgn_qk[h][:, None]
            # DR layouts: d = 256r + 128i + p
            dr = xss.reshape(DRCH, 2, P, T)
            if swi:
                # per 128-s-block: A/B interleave, columns reversed
                a = dr.reshape(DRCH, 2, P, NSC, P)   # [r, i, p, rc, mm]
                a = a[..., ::-1]                     # reverse mm
                a = a.transpose(0, 2, 3, 4, 1)       # [r, p, rc, mm_r, i]
                xst[slot] = a.reshape(DRCH, P, NSC, 2 * P).astype(F8)
            else:
                xst[slot] = dr.transpose(0, 2, 1, 3).astype(F8)
            xpt[slot] = xsT.reshape(DRCH, 2, P, T).transpose(0, 2, 1, 3).astype(F8)
            xv = xs * v_bind[h][None, :]             # [T, HD]
            xv16[slot] = xv.reshape(NSC, P, HD).astype(np.float16)
            xv8[slot] = (SV * xv).reshape(NPC, 2, P, HD).transpose(
                0, 2, 1, 3).astype(F8)
            cvec[slot] = c[h] if SIGMOID else c[h] * SDEV / 4
            if host_tril:
                # full 0.5 * inclusive cumsum bias, added in the epilogue
                prefs.append(0.5 * np.cumsum(xv, axis=0, dtype=np.float32))
            else:
                bsum = xv.reshape(NSC, P, HD).sum(axis=1)      # [NSC, HD]
                pref = np.cumsum(bsum, axis=0) - bsum          # exclusive
                prefs.append(pref.astype(np.float32))
        m = {"xst": xst, "xpt": xpt, "xv8": xv8, "cvec": cvec}
        if not host_tril:
            if tril8:
                m["tril8"] = tril8_arr
            else:
                m["xv16"] = xv16
                m["tril"] = tril16
        in_maps.append(m)
    return in_maps, prefs


def assemble_output(results, prefs):
    out = np.empty((B, T, D), np.float32)
    oh = out.reshape(B, T, H, HD)
    for core in range(N_CORES):
        for slot in range(PAIRS):
            bh = PAIRS * core + slot
            b, h = divmod(bh, H)
            raw = results[core]["out"][slot].astype(np.float32) / SNET
            if prefs[bh].shape == (T, HD):
                raw += prefs[bh]
            else:
                raw.reshape(NSC, P, HD)[:] += 0.5 * prefs[bh][:, None, :]
            oh[b, :, h, :] = raw
    return out


def kernel(x, bv_q, bv_k, bv_v):
    nc = get_program()
    in_maps, prefs = make_in_maps(x, bv_q, bv_k, bv_v)
    res = run_bass_kernel_spmd(nc, in_maps, list(range(N_CORES)))
    return assemble_output(res.results, prefs)
